# revision 17
# baseline (speedup 1.0000x reference)
"""Causal MHA (B=2, N=2048, D=1024, H=16) on 8 NeuronCores via Bass/Tile.

Sharding: core c = (b, g): b = c // 4 (batch), g = c % 4 (head group of 4
heads = 256 features). Each core computes its Q/K/V projections, causal
attention for its 4 heads, and a partial output projection (its 256 rows of
Wo). The host sums the 4 partials per batch ("unshard" of row-parallel TP).

Layout: activations are feature-major (features on SBUF partitions, sequence
on the free axis), so S^T = K Q^T tiles come out of the PE with k on
partitions and q free and exp() needs no reduction at all. The two heads of
a 128-partition pair run as concurrent 64x128 PE row tiles (bass auto-derives
tile_position from the base partitions 0/64 and the different PSUM banks).
The softmax denominator falls out of the P@V matmul via a ones column
appended to V; the per-(head, q) normalization uses a reciprocal row
broadcast across partitions through a DRAM bounce (SBUF-source partition
broadcast is not supported by the DMA AP).

The whole datapath is bf16 (inputs, weights, P, V, partial outputs; PSUM
accumulation stays fp32), which halves DMA and DVE volume vs fp32 — the
rel-err budget is 2e-2 and bf16 end-to-end lands ~4.6e-3. Projections are
pipelined per 512-wide sequence chunk (full-depth PSUM accumulation over all
8 d-tiles, bias applied during the ScalarE evacuation) and then attention for
that chunk runs, so projection DMA/PE work overlaps the ScalarE-bound softmax
of earlier chunks. The exp() only covers the causally-valid columns of each
diagonal score tile.
"""

import numpy as np
import ml_dtypes

import concourse.bass as bass
import concourse.bacc as bacc
import concourse.mybir as mybir
from concourse.tile import TileContext
from concourse.bass_utils import run_bass_kernel_spmd

F32 = mybir.dt.float32
BF16 = mybir.dt.bfloat16
AF = mybir.ActivationFunctionType
NPBF = ml_dtypes.bfloat16

B, N, D, H, DH = 2, 2048, 1024, 16, 64
NCORES = 8
GROUPS = 4
HPC = H // GROUPS     # 4 heads per core
FS = HPC * DH         # 256
P = 128
NDT = N // 128        # 16
NSS = N // 512        # 4
DT = D // 128         # 8
FT = FS // 128        # 2

_CACHE = {}


def _build(repeat=1, phases="all"):
    nc = bacc.Bacc("TRN2", target_bir_lowering=False, debug=False)

    xqT = nc.dram_tensor("xqT", [D, N], BF16, kind="ExternalInput")
    xkvT = nc.dram_tensor("xkvT", [D, N], BF16, kind="ExternalInput")
    wq = nc.dram_tensor("wq", [D, FS], BF16, kind="ExternalInput")
    wk = nc.dram_tensor("wk", [D, FS], BF16, kind="ExternalInput")
    wv = nc.dram_tensor("wv", [D, FS], BF16, kind="ExternalInput")
    wo = nc.dram_tensor("wo", [FS, D], BF16, kind="ExternalInput")
    bq = nc.dram_tensor("bq", [FS], F32, kind="ExternalInput")
    bk = nc.dram_tensor("bk", [FS], F32, kind="ExternalInput")
    bv = nc.dram_tensor("bv", [1, FS], BF16, kind="ExternalInput")
    bo = nc.dram_tensor("bo", [1, D], BF16, kind="ExternalInput")
    masks = nc.dram_tensor("masks", [P, P], BF16, kind="ExternalInput")
    out = nc.dram_tensor("out_p", [N, D], BF16, kind="ExternalOutput")

    with TileContext(nc) as tc:
        with (
            tc.tile_pool(name="const", bufs=1) as cp,
            tc.tile_pool(name="xt", bufs=2) as xp,
            tc.tile_pool(name="acts", bufs=1) as ap_,
            tc.tile_pool(name="ps", bufs=2, space="PSUM") as psp,
            tc.tile_pool(name="pt", bufs=3) as ptp,
            tc.tile_pool(name="small", bufs=4) as smp,
            tc.tile_pool(name="osb", bufs=3) as osp,
            tc.tile_pool(name="dsc", bufs=4, space="DRAM") as dsp,
        ):
            wq_sb = cp.tile([P, DT, FS], BF16, tag="wq")
            wk_sb = cp.tile([P, DT, FS], BF16, tag="wk")
            wv_sb = cp.tile([P, DT, FS], BF16, tag="wv")
            wo_sb = cp.tile([P, FT, D], BF16, tag="wo")
            bqk_sb = cp.tile([P, 2, 2], F32, tag="bqk")
            bv_sb = cp.tile([1, FS], BF16, tag="bv")
            bo_sb = cp.tile([1, D], BF16, tag="bo")
            tri_sb = cp.tile([P, P], BF16, tag="mask")
            ones_r = cp.tile([1, P], BF16, tag="ones")
            ones_f = cp.tile([P, HPC], BF16, tag="ones_f")
            bo_rep = cp.tile([P, D], F32, tag="bo_rep")
            bv_rep = cp.tile([P, FS], F32, tag="bv_rep")

            nc.sync.dma_start(out=wq_sb, in_=wq.ap().rearrange("(t p) f -> p t f", p=P))
            nc.sync.dma_start(out=wk_sb, in_=wk.ap().rearrange("(t p) f -> p t f", p=P))
            nc.sync.dma_start(out=wv_sb, in_=wv.ap().rearrange("(t p) f -> p t f", p=P))
            nc.sync.dma_start(out=wo_sb, in_=wo.ap().rearrange("(t p) f -> p t f", p=P))
            nc.sync.dma_start(out=bqk_sb[:, 0, :], in_=bk.ap().rearrange("(t p) -> p t", p=P))
            nc.sync.dma_start(out=bqk_sb[:, 1, :], in_=bq.ap().rearrange("(t p) -> p t", p=P))
            nc.sync.dma_start(out=bv_sb, in_=bv.ap())
            nc.sync.dma_start(out=bo_sb, in_=bo.ap())
            nc.sync.dma_start(out=tri_sb, in_=masks.ap())
            nc.vector.memset(ones_f, 1.0)
            nc.vector.memset(ones_r, 1.0)

            # one-time replicated bias tiles (K=1 matmul broadcast)
            ps_rep = psp.tile([P, 512], F32, tag="ps", name="ps_brep")
            nc.tensor.matmul(ps_rep, ones_r[:, 0:P], bo_sb[:, 0:512], start=True, stop=True)
            nc.vector.tensor_copy(bo_rep[:, 0:512], ps_rep)
            ps_rep2 = psp.tile([P, 512], F32, tag="ps", name="ps_brep2")
            nc.tensor.matmul(ps_rep2, ones_r[:, 0:P], bo_sb[:, 512:1024], start=True, stop=True)
            nc.vector.tensor_copy(bo_rep[:, 512:1024], ps_rep2)
            ps_rep3 = psp.tile([P, 512], F32, tag="ps", name="ps_brep3")
            nc.tensor.matmul(ps_rep3[:, 0:FS], ones_r[:, 0:P], bv_sb, start=True, stop=True)
            nc.vector.tensor_copy(bv_rep, ps_rep3[:, 0:FS])

            kt_all = [ap_.tile([P, N], BF16, tag=f"kt{f}", name=f"kt{f}") for f in range(FT)]
            qt_all = [ap_.tile([P, N], BF16, tag=f"qt{f}", name=f"qt{f}") for f in range(FT)]
            v_sb = [ap_.tile([P, HPC, DH + 1], BF16, tag=f"v{st}", name=f"v{st}") for st in range(NDT)]
            ot_all = [ap_.tile([P, N], BF16, tag=f"ot{f}", name=f"ot{f}") for f in range(FT)]

            def emit_oproj(ss_):
                for qt in range(4 * ss_, 4 * ss_ + 4):
                    o_sb = osp.tile([P, D], BF16, tag="osb", name="o_sb")
                    for os_ in range(2):
                        ps_o = psp.tile([P, 512], F32, tag="ps", name="ps_o")
                        for ft in range(FT):
                            nc.tensor.matmul(
                                ps_o,
                                ot_all[ft][:, qt * P:(qt + 1) * P],
                                wo_sb[:, ft, os_ * 512:(os_ + 1) * 512],
                                start=(ft == 0),
                                stop=(ft == FT - 1),
                            )
                        nc.vector.tensor_add(
                            o_sb[:, os_ * 512:(os_ + 1) * 512],
                            ps_o,
                            bo_rep[:, os_ * 512:(os_ + 1) * 512],
                        )
                    nc.sync.dma_start(out=out.ap()[qt * P:(qt + 1) * P, :], in_=o_sb)

            def emit_body():
                for ss in range(NSS):
                    s0 = ss * 512
                    # ---- chunk DMAs ----
                    xkv_t, xq_t = [], []
                    for d in range(DT):
                        t = xp.tile([P, 512], BF16, tag=f"xkv{d}", name=f"xkv{d}")
                        nc.sync.dma_start(out=t, in_=xkvT.ap()[d * P:(d + 1) * P, s0:s0 + 512])
                        xkv_t.append(t)
                    for d in range(DT):
                        t = xp.tile([P, 512], BF16, tag=f"xq{d}", name=f"xq{d}")
                        nc.sync.dma_start(out=t, in_=xqT.ap()[d * P:(d + 1) * P, s0:s0 + 512])
                        xq_t.append(t)

                    # ---- K chunk ----
                    for ft in range(FT):
                        ps = psp.tile([P, 512], F32, tag="ps", name="ps_k")
                        for d in range(DT):
                            nc.tensor.matmul(
                                ps,
                                wk_sb[:, d, ft * P:(ft + 1) * P],
                                xkv_t[d],
                                start=(d == 0),
                                stop=(d == DT - 1),
                            )
                        nc.scalar.activation(
                            kt_all[ft][:, s0:s0 + 512], ps, AF.Identity,
                            bias=bqk_sb[:, 0, ft:ft + 1],
                        )
                    # ---- V chunk ----
                    for st in range(4):
                        psv = psp.tile([P, 512], F32, tag="ps", name="ps_v")
                        for d in range(DT):
                            nc.tensor.matmul(
                                psv[:, 0:FS],
                                xkv_t[d][:, st * P:(st + 1) * P],
                                wv_sb[:, d, :],
                                start=(d == 0),
                                stop=(d == DT - 1),
                            )
                        v = v_sb[ss * 4 + st]
                        nc.vector.tensor_add(
                            v[:, :, 0:DH],
                            psv[:, 0:FS].rearrange("p (h c) -> p h c", h=HPC),
                            bv_rep.rearrange("p (h c) -> p h c", h=HPC),
                        )
                        nc.vector.tensor_copy(v[:, :, DH], ones_f)
                    # ---- Q chunk ----
                    for ft in range(FT):
                        ps = psp.tile([P, 512], F32, tag="ps", name="ps_q")
                        for d in range(DT):
                            nc.tensor.matmul(
                                ps,
                                wq_sb[:, d, ft * P:(ft + 1) * P],
                                xq_t[d],
                                start=(d == 0),
                                stop=(d == DT - 1),
                            )
                        nc.scalar.activation(
                            qt_all[ft][:, s0:s0 + 512], ps, AF.Identity,
                            bias=bqk_sb[:, 1, ft:ft + 1],
                        )

                    if phases == "proj":
                        continue

                    # ---- attention for chunk ss ----
                    n_kt = 4 * ss + 4
                    for ft in range(FT):
                        otp = [
                            psp.tile([P, 512], F32, tag=f"otp{hh}", bufs=1, name=f"ps_ot{hh}")
                            for hh in range(2)
                        ]
                        for kt in range(n_kt):
                            st2 = psp.tile([P, 1024], F32, tag="ps2", bufs=2, name="ps_st2")
                            ptt = ptp.tile([P, 1024], BF16, tag="pt", name="ptt")
                            for hh in range(2):
                                nc.tensor.matmul(
                                    st2[:, hh * 512:(hh + 1) * 512],
                                    kt_all[ft][hh * 64:(hh + 1) * 64, kt * P:(kt + 1) * P],
                                    qt_all[ft][hh * 64:(hh + 1) * 64, s0:s0 + 512],
                                    start=True, stop=True,
                                )
                            dk = (kt - 4 * ss) * P
                            if dk <= 0:
                                # fully-valid tile (or diagonal block at the
                                # chunk start): exp the whole thing
                                nc.scalar.activation(ptt, st2, AF.Exp, scale=0.125)
                            else:
                                # exp only the causally-valid columns of each
                                # head half; zero the dead region (PV reads it)
                                st2v = st2.rearrange("p (h q) -> p h q", h=2)
                                pttv = ptt.rearrange("p (h q) -> p h q", h=2)
                                nc.scalar.activation(
                                    pttv[:, :, dk:512], st2v[:, :, dk:512],
                                    AF.Exp, scale=0.125,
                                )
                                for hh in range(2):
                                    nc.vector.memset(ptt[:, hh * 512:hh * 512 + dk], 0.0)
                            if dk >= 0:
                                for hh in range(2):
                                    base = hh * 512 + max(dk, 0)
                                    nc.vector.tensor_mul(
                                        ptt[:, base:base + P],
                                        ptt[:, base:base + P],
                                        tri_sb,
                                    )
                            for hh in range(2):
                                nc.tensor.matmul(
                                    otp[hh][0:DH + 1, :],
                                    v_sb[kt][:, ft * 2 + hh, :],
                                    ptt[:, hh * 512:(hh + 1) * 512],
                                    start=(kt == 0),
                                    stop=(kt == n_kt - 1),
                                )
                        # normalization: reciprocal row, broadcast via DRAM bounce
                        rept = smp.tile([DH + 1, 1024], F32, tag="rep_sb", bufs=2, name="rept")
                        recip = rept[DH:DH + 1, :]
                        rep_sb = rept[0:DH, :]
                        with nc.allow_low_precision(reason="softmax reciprocal"):
                            nc.vector.reciprocal(recip[:, 0:512], otp[0][DH:DH + 1, :])
                            nc.vector.reciprocal(recip[:, 512:1024], otp[1][DH:DH + 1, :])
                        dscr = dsp.tile([1, 1024], F32, tag="dscr", name="dscr")
                        nc.sync.dma_start(out=dscr, in_=recip)
                        rep_bcast = bass.AP(
                            tensor=dscr.tensor,
                            offset=dscr.offset,
                            ap=[[0, DH]] + [list(x) for x in dscr.ap[1:]],
                        )
                        nc.sync.dma_start(out=rep_sb, in_=rep_bcast)
                        for hh in range(2):
                            row = hh * 64
                            nc.vector.tensor_mul(
                                ot_all[ft][row:row + 64, s0:s0 + 512],
                                otp[hh][0:DH, :],
                                rep_sb[:, hh * 512:(hh + 1) * 512],
                            )

                    if phases == "proj+attn":
                        continue
                    # O-proj deferred one ss so the PE never waits on the
                    # normalize chain of the slice it is about to project
                    if ss > 0:
                        emit_oproj(ss - 1)
                    if ss == NSS - 1:
                        emit_oproj(ss)

                if phases == "proj":
                    row = 0
                    for tset in (kt_all, qt_all):
                        for tt in tset:
                            for half in range(2):
                                nc.sync.dma_start(
                                    out=out.ap()[row * P:(row + 1) * P, :],
                                    in_=tt[:, half * D:(half + 1) * D],
                                )
                                row += 1
                    for st in range(NDT):
                        rr = 8 + st % 8
                        nc.sync.dma_start(
                            out=out.ap()[rr * P:(rr + 1) * P, 0:HPC * (DH + 1)],
                            in_=v_sb[st].rearrange("p h c -> p (h c)"),
                        )
                elif phases == "proj+attn":
                    row = 0
                    for tt in ot_all:
                        for half in range(2):
                            nc.sync.dma_start(
                                out=out.ap()[row * P:(row + 1) * P, :],
                                in_=tt[:, half * D:(half + 1) * D],
                            )
                            row += 1

            if repeat == 1:
                emit_body()
            else:
                with tc.For_i(0, repeat, 1):
                    emit_body()

    nc.compile()
    return nc


def _shard_inputs(x_q, x_kv, Wq, bq_, Wk, bk_, Wv, bv_, Wo, bo_):
    pp_, ff = np.meshgrid(np.arange(P), np.arange(P), indexing="ij")
    mask = (ff >= pp_).astype(NPBF)
    in_maps = []
    for c in range(NCORES):
        b, g = c // GROUPS, c % GROUPS
        sl = slice(g * FS, (g + 1) * FS)
        in_maps.append({
            "xqT": np.ascontiguousarray(x_q[b].T).astype(NPBF),
            "xkvT": np.ascontiguousarray(x_kv[b].T).astype(NPBF),
            "wq": np.ascontiguousarray(Wq[:, sl]).astype(NPBF),
            "wk": np.ascontiguousarray(Wk[:, sl]).astype(NPBF),
            "wv": np.ascontiguousarray(Wv[:, sl]).astype(NPBF),
            "wo": np.ascontiguousarray(Wo[sl, :]).astype(NPBF),
            "bq": np.ascontiguousarray(bq_[sl]),
            "bk": np.ascontiguousarray(bk_[sl]),
            "bv": np.ascontiguousarray(bv_[sl]).reshape(1, FS).astype(NPBF),
            "bo": (bo_ if g == 0 else np.zeros_like(bo_)).reshape(1, D).astype(NPBF),
            "masks": mask,
        })
    return in_maps


def kernel(x_q, x_kv, Wq, bq, Wk, bk, Wv, bv, Wo, bo):
    x_q = np.asarray(x_q, dtype=np.float32)
    x_kv = np.asarray(x_kv, dtype=np.float32)
    if "nc" not in _CACHE:
        _CACHE["nc"] = _build()
    nc = _CACHE["nc"]
    in_maps = _shard_inputs(
        x_q, x_kv,
        np.asarray(Wq, np.float32), np.asarray(bq, np.float32),
        np.asarray(Wk, np.float32), np.asarray(bk, np.float32),
        np.asarray(Wv, np.float32), np.asarray(bv, np.float32),
        np.asarray(Wo, np.float32), np.asarray(bo, np.float32),
    )
    res = run_bass_kernel_spmd(nc, in_maps, core_ids=list(range(NCORES)))
    out = np.zeros((B, N, D), dtype=np.float32)
    for c in range(NCORES):
        out[c // GROUPS] += np.asarray(res.results[c]["out_p"], dtype=np.float32)
    return out


# revision 26
# speedup vs baseline: 1.0238x; 1.0238x over previous
"""Causal MHA (B=2, N=2048, D=1024, H=16) on 8 NeuronCores via Bass/Tile.

Sharding: core c = (b, g): b = c // 4 (batch), g = c % 4 (head group of 4
heads = 256 features). Each core computes its Q/K/V projections, causal
attention for its 4 heads, and a partial output projection (its 256 rows of
Wo). The host sums the 4 partials per batch ("unshard" of row-parallel TP).

Layout: activations are feature-major (features on SBUF partitions, sequence
on the free axis), so S^T = K Q^T tiles come out of the PE with k on
partitions and q free and exp() needs no reduction at all. The two heads of
a 128-partition pair run as concurrent 64x128 PE row tiles (bass auto-derives
tile_position from the base partitions 0/64 and the different PSUM banks).
The softmax denominator falls out of the P@V matmul via a ones column
appended to V; the per-(head, q) normalization uses a reciprocal row
broadcast across partitions through a DRAM bounce (SBUF-source partition
broadcast is not supported by the DMA AP).

The whole datapath is bf16 (inputs, weights, P, V, partial outputs; PSUM
accumulation stays fp32), which halves DMA and DVE volume vs fp32 — the
rel-err budget is 2e-2 and bf16 end-to-end lands ~4.6e-3. Projections are
pipelined per 512-wide sequence chunk (full-depth PSUM accumulation over all
8 d-tiles, bias applied during the ScalarE evacuation) and then attention for
that chunk runs, so projection DMA/PE work overlaps the ScalarE-bound softmax
of earlier chunks. Score matmuls, exp() and the P@V matmuls all cover only
the causally-valid trapezoid of each diagonal tile. The reciprocal-broadcast
bounce DMAs ride the Activation HWDGE queue so they never wait behind the
bulk x/output transfers on the SP queue (measured -11us); putting bulk
transfers on the Activation queue regresses (the triggers stall the exp
stream), so only the two tiny bounce DMAs go there.
"""

import numpy as np
import ml_dtypes

import concourse.bass as bass
import concourse.bacc as bacc
import concourse.mybir as mybir
from concourse.tile import TileContext
from concourse.bass_utils import run_bass_kernel_spmd

F32 = mybir.dt.float32
BF16 = mybir.dt.bfloat16
AF = mybir.ActivationFunctionType
NPBF = ml_dtypes.bfloat16

B, N, D, H, DH = 2, 2048, 1024, 16, 64
NCORES = 8
GROUPS = 4
HPC = H // GROUPS     # 4 heads per core
FS = HPC * DH         # 256
P = 128
NDT = N // 128        # 16
NSS = N // 512        # 4
DT = D // 128         # 8
FT = FS // 128        # 2

_CACHE = {}


def _build(repeat=1, phases="all"):
    nc = bacc.Bacc("TRN2", target_bir_lowering=False, debug=False)

    xqT = nc.dram_tensor("xqT", [D, N], BF16, kind="ExternalInput")
    xkvT = nc.dram_tensor("xkvT", [D, N], BF16, kind="ExternalInput")
    wq = nc.dram_tensor("wq", [D, FS], BF16, kind="ExternalInput")
    wk = nc.dram_tensor("wk", [D, FS], BF16, kind="ExternalInput")
    wv = nc.dram_tensor("wv", [D, FS], BF16, kind="ExternalInput")
    wo = nc.dram_tensor("wo", [FS, D], BF16, kind="ExternalInput")
    bq = nc.dram_tensor("bq", [FS], F32, kind="ExternalInput")
    bk = nc.dram_tensor("bk", [FS], F32, kind="ExternalInput")
    bv = nc.dram_tensor("bv", [1, FS], BF16, kind="ExternalInput")
    bo = nc.dram_tensor("bo", [1, D], BF16, kind="ExternalInput")
    masks = nc.dram_tensor("masks", [P, P], BF16, kind="ExternalInput")
    out = nc.dram_tensor("out_p", [N, D], BF16, kind="ExternalOutput")

    with TileContext(nc) as tc:
        with (
            tc.tile_pool(name="const", bufs=1) as cp,
            tc.tile_pool(name="xt", bufs=2) as xp,
            tc.tile_pool(name="acts", bufs=1) as ap_,
            tc.tile_pool(name="ps", bufs=2, space="PSUM") as psp,
            tc.tile_pool(name="pt", bufs=4) as ptp,
            tc.tile_pool(name="small", bufs=4) as smp,
            tc.tile_pool(name="osb", bufs=3) as osp,
            tc.tile_pool(name="dsc", bufs=4, space="DRAM") as dsp,
        ):
            wq_sb = cp.tile([P, DT, FS], BF16, tag="wq")
            wk_sb = cp.tile([P, DT, FS], BF16, tag="wk")
            wv_sb = cp.tile([P, DT, FS], BF16, tag="wv")
            wo_sb = cp.tile([P, FT, D], BF16, tag="wo")
            bqk_sb = cp.tile([P, 2, 2], F32, tag="bqk")
            bv_sb = cp.tile([1, FS], BF16, tag="bv")
            bo_sb = cp.tile([1, D], BF16, tag="bo")
            tri_sb = cp.tile([P, P], BF16, tag="mask")
            ones_r = cp.tile([1, P], BF16, tag="ones")
            ones_f = cp.tile([P, HPC], BF16, tag="ones_f")
            bo_rep = cp.tile([P, D], F32, tag="bo_rep")
            bv_rep = cp.tile([P, FS], F32, tag="bv_rep")

            nc.sync.dma_start(out=wq_sb, in_=wq.ap().rearrange("(t p) f -> p t f", p=P))
            nc.sync.dma_start(out=wk_sb, in_=wk.ap().rearrange("(t p) f -> p t f", p=P))
            nc.sync.dma_start(out=wv_sb, in_=wv.ap().rearrange("(t p) f -> p t f", p=P))
            nc.sync.dma_start(out=wo_sb, in_=wo.ap().rearrange("(t p) f -> p t f", p=P))
            nc.sync.dma_start(out=bqk_sb[:, 0, :], in_=bk.ap().rearrange("(t p) -> p t", p=P))
            nc.sync.dma_start(out=bqk_sb[:, 1, :], in_=bq.ap().rearrange("(t p) -> p t", p=P))
            nc.sync.dma_start(out=bv_sb, in_=bv.ap())
            nc.sync.dma_start(out=bo_sb, in_=bo.ap())
            nc.sync.dma_start(out=tri_sb, in_=masks.ap())
            nc.vector.memset(ones_f, 1.0)
            nc.vector.memset(ones_r, 1.0)

            # one-time replicated bias tiles (K=1 matmul broadcast)
            ps_rep = psp.tile([P, 512], F32, tag="ps", name="ps_brep")
            nc.tensor.matmul(ps_rep, ones_r[:, 0:P], bo_sb[:, 0:512], start=True, stop=True)
            nc.vector.tensor_copy(bo_rep[:, 0:512], ps_rep)
            ps_rep2 = psp.tile([P, 512], F32, tag="ps", name="ps_brep2")
            nc.tensor.matmul(ps_rep2, ones_r[:, 0:P], bo_sb[:, 512:1024], start=True, stop=True)
            nc.vector.tensor_copy(bo_rep[:, 512:1024], ps_rep2)
            ps_rep3 = psp.tile([P, 512], F32, tag="ps", name="ps_brep3")
            nc.tensor.matmul(ps_rep3[:, 0:FS], ones_r[:, 0:P], bv_sb, start=True, stop=True)
            nc.vector.tensor_copy(bv_rep, ps_rep3[:, 0:FS])

            kt_all = [ap_.tile([P, N], BF16, tag=f"kt{f}", name=f"kt{f}") for f in range(FT)]
            qt_all = [ap_.tile([P, N], BF16, tag=f"qt{f}", name=f"qt{f}") for f in range(FT)]
            v_sb = [ap_.tile([P, HPC, DH + 1], BF16, tag=f"v{st}", name=f"v{st}") for st in range(NDT)]
            ot_all = [ap_.tile([P, N], BF16, tag=f"ot{f}", name=f"ot{f}") for f in range(FT)]

            # ones column of V written once; the V evacuation never touches it
            for st in range(NDT):
                nc.vector.tensor_copy(v_sb[st][:, :, DH], ones_f)

            def emit_oproj(ss_):
                for qt in range(4 * ss_, 4 * ss_ + 4):
                    o_sb = osp.tile([P, D], BF16, tag="osb", name="o_sb")
                    for os_ in range(2):
                        ps_o = psp.tile([P, 512], F32, tag="ps", name="ps_o")
                        for ft in range(FT):
                            nc.tensor.matmul(
                                ps_o,
                                ot_all[ft][:, qt * P:(qt + 1) * P],
                                wo_sb[:, ft, os_ * 512:(os_ + 1) * 512],
                                start=(ft == 0),
                                stop=(ft == FT - 1),
                            )
                        nc.vector.tensor_add(
                            o_sb[:, os_ * 512:(os_ + 1) * 512],
                            ps_o,
                            bo_rep[:, os_ * 512:(os_ + 1) * 512],
                        )
                    nc.sync.dma_start(out=out.ap()[qt * P:(qt + 1) * P, :], in_=o_sb)

            def emit_body():
                for ss in range(NSS):
                    s0 = ss * 512
                    # ---- chunk DMAs ----
                    xkv_t, xq_t = [], []
                    for d in range(DT):
                        t = xp.tile([P, 512], BF16, tag=f"xkv{d}", name=f"xkv{d}")
                        nc.sync.dma_start(out=t, in_=xkvT.ap()[d * P:(d + 1) * P, s0:s0 + 512])
                        xkv_t.append(t)
                    for d in range(DT):
                        t = xp.tile([P, 512], BF16, tag=f"xq{d}", name=f"xq{d}")
                        nc.sync.dma_start(out=t, in_=xqT.ap()[d * P:(d + 1) * P, s0:s0 + 512])
                        xq_t.append(t)

                    # ---- K chunk ----
                    for ft in range(FT):
                        ps = psp.tile([P, 512], F32, tag="ps", name="ps_k")
                        for d in range(DT):
                            nc.tensor.matmul(
                                ps,
                                wk_sb[:, d, ft * P:(ft + 1) * P],
                                xkv_t[d],
                                start=(d == 0),
                                stop=(d == DT - 1),
                            )
                        nc.scalar.activation(
                            kt_all[ft][:, s0:s0 + 512], ps, AF.Identity,
                            bias=bqk_sb[:, 0, ft:ft + 1],
                        )
                    # ---- V chunk ----
                    for st in range(4):
                        psv = psp.tile([P, 512], F32, tag="ps", name="ps_v")
                        for d in range(DT):
                            nc.tensor.matmul(
                                psv[:, 0:FS],
                                xkv_t[d][:, st * P:(st + 1) * P],
                                wv_sb[:, d, :],
                                start=(d == 0),
                                stop=(d == DT - 1),
                            )
                        v = v_sb[ss * 4 + st]
                        nc.vector.tensor_add(
                            v[:, :, 0:DH],
                            psv[:, 0:FS].rearrange("p (h c) -> p h c", h=HPC),
                            bv_rep.rearrange("p (h c) -> p h c", h=HPC),
                        )
                    # ---- Q chunk ----
                    for ft in range(FT):
                        ps = psp.tile([P, 512], F32, tag="ps", name="ps_q")
                        for d in range(DT):
                            nc.tensor.matmul(
                                ps,
                                wq_sb[:, d, ft * P:(ft + 1) * P],
                                xq_t[d],
                                start=(d == 0),
                                stop=(d == DT - 1),
                            )
                        nc.scalar.activation(
                            qt_all[ft][:, s0:s0 + 512], ps, AF.Identity,
                            bias=bqk_sb[:, 1, ft:ft + 1],
                        )

                    if phases == "proj":
                        continue

                    # ---- attention for chunk ss ----
                    n_kt = 4 * ss + 4
                    for ft in range(FT):
                        otp = [
                            psp.tile([P, 512], F32, tag=f"otp{hh}", bufs=1, name=f"ps_ot{hh}")
                            for hh in range(2)
                        ]
                        for kt in range(n_kt):
                            st2 = psp.tile([P, 1024], F32, tag="ps2", bufs=2, name="ps_st2")
                            ptt = ptp.tile([P, 1024], BF16, tag="pt", name="ptt")
                            dk = max((kt - 4 * ss) * P, 0)
                            for hh in range(2):
                                nc.tensor.matmul(
                                    st2[:, hh * 512 + dk:(hh + 1) * 512],
                                    kt_all[ft][hh * 64:(hh + 1) * 64, kt * P:(kt + 1) * P],
                                    qt_all[ft][hh * 64:(hh + 1) * 64, s0 + dk:s0 + 512],
                                    start=True, stop=True,
                                )
                            if dk == 0:
                                nc.scalar.activation(ptt, st2, AF.Exp, scale=0.125)
                            else:
                                # exp only the causally-valid columns of each
                                # head half (P@V reads the same trapezoid)
                                st2v = st2.rearrange("p (h q) -> p h q", h=2)
                                pttv = ptt.rearrange("p (h q) -> p h q", h=2)
                                nc.scalar.activation(
                                    pttv[:, :, dk:512], st2v[:, :, dk:512],
                                    AF.Exp, scale=0.125,
                                )
                            if kt - 4 * ss >= 0:
                                for hh in range(2):
                                    base = hh * 512 + dk
                                    nc.vector.tensor_mul(
                                        ptt[:, base:base + P],
                                        ptt[:, base:base + P],
                                        tri_sb,
                                    )
                            for hh in range(2):
                                nc.tensor.matmul(
                                    otp[hh][0:DH + 1, dk:512],
                                    v_sb[kt][:, ft * 2 + hh, :],
                                    ptt[:, hh * 512 + dk:(hh + 1) * 512],
                                    start=(kt == 0),
                                    stop=(kt == n_kt - 1),
                                    skip_group_check=True,
                                )
                        # normalization: reciprocal row, broadcast via DRAM bounce
                        rept = smp.tile([DH + 1, 1024], F32, tag="rep_sb", bufs=4, name="rept")
                        recip = rept[DH:DH + 1, :]
                        rep_sb = rept[0:DH, :]
                        with nc.allow_low_precision(reason="softmax reciprocal"):
                            nc.vector.reciprocal(recip[:, 0:512], otp[0][DH:DH + 1, :])
                            nc.vector.reciprocal(recip[:, 512:1024], otp[1][DH:DH + 1, :])
                        # bounce rides the Activation HWDGE queue so it never
                        # waits behind the bulk x/out transfers on the SP queue
                        dscr = dsp.tile([1, 1024], F32, tag="dscr", name="dscr")
                        nc.scalar.dma_start(out=dscr, in_=recip)
                        rep_bcast = bass.AP(
                            tensor=dscr.tensor,
                            offset=dscr.offset,
                            ap=[[0, DH]] + [list(x) for x in dscr.ap[1:]],
                        )
                        nc.scalar.dma_start(out=rep_sb, in_=rep_bcast)
                        for hh in range(2):
                            row = hh * 64
                            nc.vector.tensor_mul(
                                ot_all[ft][row:row + 64, s0:s0 + 512],
                                otp[hh][0:DH, :],
                                rep_sb[:, hh * 512:(hh + 1) * 512],
                            )

                    if phases == "proj+attn":
                        continue
                    # O-proj deferred one ss so the PE never waits on the
                    # normalize chain of the slice it is about to project
                    if ss > 0:
                        emit_oproj(ss - 1)
                    if ss == NSS - 1:
                        emit_oproj(ss)

                if phases == "proj":
                    row = 0
                    for tset in (kt_all, qt_all):
                        for tt in tset:
                            for half in range(2):
                                nc.sync.dma_start(
                                    out=out.ap()[row * P:(row + 1) * P, :],
                                    in_=tt[:, half * D:(half + 1) * D],
                                )
                                row += 1
                    for st in range(NDT):
                        rr = 8 + st % 8
                        nc.sync.dma_start(
                            out=out.ap()[rr * P:(rr + 1) * P, 0:HPC * (DH + 1)],
                            in_=v_sb[st].rearrange("p h c -> p (h c)"),
                        )
                elif phases == "proj+attn":
                    row = 0
                    for tt in ot_all:
                        for half in range(2):
                            nc.sync.dma_start(
                                out=out.ap()[row * P:(row + 1) * P, :],
                                in_=tt[:, half * D:(half + 1) * D],
                            )
                            row += 1

            if repeat == 1:
                emit_body()
            else:
                with tc.For_i(0, repeat, 1):
                    emit_body()

    nc.compile()
    return nc


def _shard_inputs(x_q, x_kv, Wq, bq_, Wk, bk_, Wv, bv_, Wo, bo_):
    pp_, ff = np.meshgrid(np.arange(P), np.arange(P), indexing="ij")
    mask = (ff >= pp_).astype(NPBF)
    in_maps = []
    for c in range(NCORES):
        b, g = c // GROUPS, c % GROUPS
        sl = slice(g * FS, (g + 1) * FS)
        in_maps.append({
            "xqT": np.ascontiguousarray(x_q[b].T).astype(NPBF),
            "xkvT": np.ascontiguousarray(x_kv[b].T).astype(NPBF),
            "wq": np.ascontiguousarray(Wq[:, sl]).astype(NPBF),
            "wk": np.ascontiguousarray(Wk[:, sl]).astype(NPBF),
            "wv": np.ascontiguousarray(Wv[:, sl]).astype(NPBF),
            "wo": np.ascontiguousarray(Wo[sl, :]).astype(NPBF),
            "bq": np.ascontiguousarray(bq_[sl]),
            "bk": np.ascontiguousarray(bk_[sl]),
            "bv": np.ascontiguousarray(bv_[sl]).reshape(1, FS).astype(NPBF),
            "bo": (bo_ if g == 0 else np.zeros_like(bo_)).reshape(1, D).astype(NPBF),
            "masks": mask,
        })
    return in_maps


def kernel(x_q, x_kv, Wq, bq, Wk, bk, Wv, bv, Wo, bo):
    x_q = np.asarray(x_q, dtype=np.float32)
    x_kv = np.asarray(x_kv, dtype=np.float32)
    if "nc" not in _CACHE:
        _CACHE["nc"] = _build()
    nc = _CACHE["nc"]
    in_maps = _shard_inputs(
        x_q, x_kv,
        np.asarray(Wq, np.float32), np.asarray(bq, np.float32),
        np.asarray(Wk, np.float32), np.asarray(bk, np.float32),
        np.asarray(Wv, np.float32), np.asarray(bv, np.float32),
        np.asarray(Wo, np.float32), np.asarray(bo, np.float32),
    )
    res = run_bass_kernel_spmd(nc, in_maps, core_ids=list(range(NCORES)))
    out = np.zeros((B, N, D), dtype=np.float32)
    for c in range(NCORES):
        out[c // GROUPS] += np.asarray(res.results[c]["out_p"], dtype=np.float32)
    return out
